# revision 7
# baseline (speedup 1.0000x reference)
"""Biaffine scorer kernel for Trainium2 (Bass/Tile), data-parallel over batch
across 8 NeuronCores.

Reference computation (per batch item b):
    h = leaky_relu(state @ head_w + head_b)          # (S, BS)
    t = leaky_relu(state @ tail_w + tail_b)          # (S, BS)
    scores1[x,y,o] = h[x] @ U[o] @ t[y]
    scores2[x,y,o] = Wh·h1[x] + Wt·t1[y] + Ww·wemb[x,y] + cls_b
    out = scores1 + scores2                          # (S, S, O)

Device-side decomposition (everything feature-major so the contraction dim
sits on SBUF partitions; S padded 255->256 so fp32r matmuls stream at full
rate):

    h1T/t1T [121, 256]  = leaky(head_w.T @ stateT) (+ ones row)
    tUT [121, 2560]     : per o, [U(o).T | Wt(o)] @ t1T  -> strided-copied to
                          columns o::10; row 120 carries B[y,o]=Wt·t1[y]
    A_T [10, 256]       = WhT.T @ h1T
    out[x, (y,o)]       = h1T.T @ tUT  (+ A via indicator matmul, 2nd PSUM
                          pass) (+ C table via DVE add during PSUM->SBUF
                          evacuation)

where C[x, y*10+o] = width_table[pos(x,y)] @ Ww.T + cls_b is precomputed on
host (tiny: (256,20)@(20,10)), and the output [x, (y,o)] layout makes the
final DMA fully contiguous.
"""

import os
import numpy as np

import concourse.bass as bass
import concourse.bacc as bacc
import concourse.tile as tile
from concourse import mybir
from concourse.bass_utils import run_bass_kernel_spmd

# problem shape (hardcoded per harness contract)
B, S, H = 32, 255, 1024
BS, WD, O = 120, 20, 10
HSZ = 2 * (BS + 1) + WD
SP = 256            # padded S
NW = SP * O         # 2560
NREAL = S * O       # 2550
KT = H // 128       # 8
NCORES = 8
BPC = B // NCORES   # 4 batch items per core

F32 = mybir.dt.float32
F32R = mybir.dt.float32r

_CACHE: dict = {}


def _r(ap):
    """matmul-operand view in fp32r (full-rate) mode. Tiles feeding matmuls
    are already float32r; this is now an identity helper kept for clarity."""
    return ap if ap.dtype == F32R else ap.bitcast(F32R)


def _emit(tc, d):
    """Emit the per-core program. d: dict of DRAM APs."""
    from contextlib import ExitStack

    nc = tc.nc
    AF = mybir.ActivationFunctionType
    ALU = mybir.AluOpType

    with ExitStack() as ctx:
        const = ctx.enter_context(tc.tile_pool(name="const", bufs=1))
        st_pool = ctx.enter_context(tc.tile_pool(name="st", bufs=2))
        ht_pool = ctx.enter_context(tc.tile_pool(name="ht", bufs=2))
        tut_pool = ctx.enter_context(tc.tile_pool(name="tut", bufs=2))
        at_pool = ctx.enter_context(tc.tile_pool(name="at", bufs=2))
        out_pool = ctx.enter_context(tc.tile_pool(name="outp", bufs=3))
        pp_ht = ctx.enter_context(tc.tile_pool(name="pp_ht", bufs=1, space="PSUM"))
        pp_u = ctx.enter_context(tc.tile_pool(name="pp_u", bufs=2, space="PSUM"))
        pp_a = ctx.enter_context(tc.tile_pool(name="pp_a", bufs=1, space="PSUM"))
        pp_s = ctx.enter_context(tc.tile_pool(name="pp_s", bufs=3, space="PSUM"))

        # ---- persistent constants ----
        # head/tail weights carry an extra zero column (-> psum row 120 = 0),
        # and bias row 120 = 1.0, so the activation emits the ones row itself.
        sb_hw = const.tile([128, KT * (BS + 1)], F32R)
        nc.sync.dma_start(sb_hw[:], d["hw"])
        sb_tw = const.tile([128, KT * (BS + 1)], F32R)
        nc.sync.dma_start(sb_tw[:], d["tw"])
        sb_ut = const.tile([121, O * 121], F32R)
        nc.sync.dma_start(sb_ut[:], d["ut"])
        sb_whT = const.tile([121, O], F32R)
        nc.sync.dma_start(sb_whT[:], d["whT"])
        sb_ind = const.tile([O, NW], F32R)
        nc.sync.dma_start(sb_ind[:], d["ind"])
        sb_c0 = const.tile([128, NW], F32)
        nc.sync.dma_start(sb_c0[:], d["cmat"][0:128, :])
        sb_c1 = const.tile([127, NW], F32)
        nc.sync.dma_start(sb_c1[:], d["cmat"][128:255, :])
        sb_hb = const.tile([BS + 1, 1], F32)
        nc.sync.dma_start(sb_hb[:], d["hb"])
        sb_tb = const.tile([BS + 1, 1], F32)
        nc.sync.dma_start(sb_tb[:], d["tb"])

        for b in range(BPC):
            # ---- load stateT[b] as [128, (kt, y)] ----
            sb_sT = st_pool.tile([128, KT * SP], F32R)
            nc.sync.dma_start(
                sb_sT[:].rearrange("p (kt y) -> p kt y", kt=KT),
                d["stateT"][b].transpose([1, 0, 2]),
            )

            # ---- head/tail projections -> h1T/t1T [121, 256] ----
            ps_h = pp_ht.tile([BS + 1, SP], F32)
            ps_t = pp_ht.tile([BS + 1, SP], F32)
            for kt in range(KT):
                nc.tensor.matmul(
                    ps_h[:],
                    lhsT=_r(sb_hw[:, kt * (BS + 1):(kt + 1) * (BS + 1)]),
                    rhs=_r(sb_sT[:, kt * SP:(kt + 1) * SP]),
                    start=(kt == 0),
                    stop=(kt == KT - 1),
                )
            for kt in range(KT):
                nc.tensor.matmul(
                    ps_t[:],
                    lhsT=_r(sb_tw[:, kt * (BS + 1):(kt + 1) * (BS + 1)]),
                    rhs=_r(sb_sT[:, kt * SP:(kt + 1) * SP]),
                    start=(kt == 0),
                    stop=(kt == KT - 1),
                )
            h1T = ht_pool.tile([BS + 1, SP], F32R)
            t1T = ht_pool.tile([BS + 1, SP], F32R)
            # u = psum + bias (row 120: 0 + 1.0) ; leaky = max(u, 0.01u)
            nc.scalar.activation(h1T[:], ps_h[:], AF.Identity, bias=sb_hb[:])
            nc.vector.scalar_tensor_tensor(
                h1T[:], h1T[:], 0.01, h1T[:],
                op0=ALU.mult, op1=ALU.max,
            )
            nc.scalar.activation(t1T[:], ps_t[:], AF.Identity, bias=sb_tb[:])
            nc.vector.scalar_tensor_tensor(
                t1T[:], t1T[:], 0.01, t1T[:],
                op0=ALU.mult, op1=ALU.max,
            )

            # ---- tUT [121, 2560]: interleaved per-o copies ----
            tUT = tut_pool.tile([BS + 1, NW], F32R)
            for o in range(O):
                ps_u = pp_u.tile([121, SP], F32)
                nc.tensor.matmul(
                    ps_u[:],
                    lhsT=_r(sb_ut[:, o * 121:(o + 1) * 121]),
                    rhs=_r(t1T[:]),
                    start=True,
                    stop=True,
                )
                nc.scalar.activation(tUT[:, o:NW:O], ps_u[:], AF.Copy)

            # ---- A_T [10, 256] ----
            ps_a = pp_a.tile([O, SP], F32)
            nc.tensor.matmul(
                ps_a[:], lhsT=_r(sb_whT[:]), rhs=_r(h1T[:]), start=True, stop=True
            )
            sb_at = at_pool.tile([O, SP], F32R)
            nc.vector.tensor_copy(sb_at[:], ps_a[:])

            # ---- finals: out[x, (y,o)] per x-tile ----
            for xt in range(2):
                rows = 128 if xt == 0 else S - 128
                sb_c = sb_c0 if xt == 0 else sb_c1
                sb_out = out_pool.tile([128, NW], F32)
                for c in range(5):
                    ps_s = pp_s.tile([128, 512], F32)
                    nc.tensor.matmul(
                        ps_s[:],
                        lhsT=_r(h1T[:, xt * 128:(xt + 1) * 128]),
                        rhs=_r(tUT[:, c * 512:(c + 1) * 512]),
                        start=True,
                        stop=False,
                    )
                    nc.tensor.matmul(
                        ps_s[:],
                        lhsT=_r(sb_at[:, xt * 128:(xt + 1) * 128]),
                        rhs=_r(sb_ind[:, c * 512:(c + 1) * 512]),
                        start=False,
                        stop=True,
                    )
                    nc.vector.tensor_add(
                        sb_out[0:rows, c * 512:(c + 1) * 512],
                        ps_s[0:rows, :],
                        sb_c[0:rows, c * 512:(c + 1) * 512],
                    )
                nc.sync.dma_start(
                    d["out"][b, xt * 128:xt * 128 + rows, :],
                    sb_out[0:rows, 0:NREAL],
                )


def build_nc():
    if "nc" in _CACHE:
        return _CACHE["nc"]
    nc = bacc.Bacc(
        "TRN2", target_bir_lowering=False, debug=False, num_devices=NCORES
    )
    d = {}
    d["stateT"] = nc.dram_tensor(
        "stateT", [BPC, KT, 128, SP], F32R, kind="ExternalInput"
    ).ap()
    d["hw"] = nc.dram_tensor("hw", [128, KT * (BS + 1)], F32R, kind="ExternalInput").ap()
    d["tw"] = nc.dram_tensor("tw", [128, KT * (BS + 1)], F32R, kind="ExternalInput").ap()
    d["ut"] = nc.dram_tensor("ut", [121, O * 121], F32R, kind="ExternalInput").ap()
    d["whT"] = nc.dram_tensor("whT", [121, O], F32R, kind="ExternalInput").ap()
    d["ind"] = nc.dram_tensor("ind", [O, NW], F32R, kind="ExternalInput").ap()
    d["cmat"] = nc.dram_tensor("cmat", [S, NW], F32, kind="ExternalInput").ap()
    d["hb"] = nc.dram_tensor("hb", [BS + 1, 1], F32, kind="ExternalInput").ap()
    d["tb"] = nc.dram_tensor("tb", [BS + 1, 1], F32, kind="ExternalInput").ap()
    d["out"] = nc.dram_tensor("out", [BPC, S, NREAL], F32, kind="ExternalOutput").ap()

    # stateT[b] AP used as [KT, 128, SP] -> transposed to [128, KT, SP] at use
    d["stateT"] = d["stateT"].rearrange("b kt p y -> b (kt p) y").rearrange(
        "b (kt p) y -> b kt p y", p=128
    )

    with tile.TileContext(nc) as tc:
        _emit(tc, d)
    nc.compile()
    _CACHE["nc"] = nc
    return nc


def prep_inputs(inputs):
    """Host-side constant packing + state transpose. Returns dict of np arrays
    shared across cores (stateT is full-batch; shard before dispatch)."""
    state = np.asarray(inputs["state"], np.float32)
    head_w = np.asarray(inputs["head_w"], np.float32)
    head_b = np.asarray(inputs["head_b"], np.float32)
    tail_w = np.asarray(inputs["tail_w"], np.float32)
    tail_b = np.asarray(inputs["tail_b"], np.float32)
    U = np.asarray(inputs["U"], np.float32)
    width_table = np.asarray(inputs["width_table"], np.float32)
    cls_w = np.asarray(inputs["cls_w"], np.float32)
    cls_b = np.asarray(inputs["cls_b"], np.float32)

    stateT = np.zeros((B, H, SP), np.float32)
    stateT[:, :, :S] = state.transpose(0, 2, 1)
    stateT = np.ascontiguousarray(stateT.reshape(B, KT, 128, SP))

    hw_sb = np.zeros((128, KT, BS + 1), np.float32)
    hw_sb[:, :, :BS] = head_w.reshape(KT, 128, BS).transpose(1, 0, 2)
    hw_sb = np.ascontiguousarray(hw_sb.reshape(128, KT * (BS + 1)))
    tw_sb = np.zeros((128, KT, BS + 1), np.float32)
    tw_sb[:, :, :BS] = tail_w.reshape(KT, 128, BS).transpose(1, 0, 2)
    tw_sb = np.ascontiguousarray(tw_sb.reshape(128, KT * (BS + 1)))

    ut_ext = np.zeros((121, O, 121), np.float32)
    ut_ext[:BS, :, :BS] = U.transpose(2, 0, 1)            # [j, o, i] = U[o,i,j]
    ut_ext[:, :, BS] = cls_w[:, BS + 1:2 * (BS + 1)].T    # Wt (incl ones coeff)
    ut_ext = np.ascontiguousarray(ut_ext.reshape(121, O * 121))

    whT = np.ascontiguousarray(cls_w[:, :BS + 1].T)       # [121, 10]
    ind = np.ascontiguousarray(np.tile(np.eye(O, dtype=np.float32), (1, SP)))

    pos = np.arange(S)[None, :] - np.arange(S)[:, None] + 1
    pos = pos * (pos > 0)
    wproj = width_table @ cls_w[:, 2 * (BS + 1):].T + cls_b   # [256, 10]
    cmat = np.zeros((S, NW), np.float32)
    cmat[:, :NREAL] = wproj[pos].reshape(S, NREAL)

    return {
        "stateT": stateT,
        "hw": hw_sb,
        "tw": tw_sb,
        "ut": ut_ext,
        "whT": whT,
        "ind": ind,
        "cmat": cmat,
        "hb": np.ascontiguousarray(np.concatenate([head_b, [1.0]]).astype(np.float32)[:, None]),
        "tb": np.ascontiguousarray(np.concatenate([tail_b, [1.0]]).astype(np.float32)[:, None]),
    }


def run(inputs, trace=False, trace_kwargs=None):
    nc = build_nc()
    full = prep_inputs(inputs)
    shared = {k: v for k, v in full.items() if k != "stateT"}
    in_maps = []
    for c in range(NCORES):
        m = dict(shared)
        m["stateT"] = np.ascontiguousarray(full["stateT"][c * BPC:(c + 1) * BPC])
        in_maps.append(m)
    res = run_bass_kernel_spmd(
        nc,
        in_maps,
        core_ids=list(range(NCORES)),
        trace=trace,
        **(trace_kwargs or {}),
    )
    out = np.concatenate([r["out"] for r in res.results], axis=0)
    out = out.reshape(B, S, S, O)
    return out, res


def kernel(**inputs):
    out, _ = run(inputs, trace=False)
    return out


if __name__ == "__main__":
    # smoke: build only
    build_nc()
    print("build ok")


# revision 23
# speedup vs baseline: 3.6233x; 3.6233x over previous
"""Biaffine scorer kernel for Trainium2 (Bass/Tile), data-parallel over batch
across 8 NeuronCores.

Reference computation (per batch item b):
    h = leaky_relu(state @ head_w + head_b)          # (S, BS)
    t = leaky_relu(state @ tail_w + tail_b)          # (S, BS)
    scores1[x,y,o] = h[x] @ U[o] @ t[y]
    scores2[x,y,o] = Wh·h1[x] + Wt·t1[y] + Ww·wemb[x,y] + cls_b
    out = scores1 + scores2                          # (S, S, O)

Device-side decomposition. Everything is feature-major (contraction dim on
SBUF partitions); S padded 255->256; batch items processed in PAIRS so every
matmul streams N=512 columns (fp32r full rate, LDWEIGHTS fully hidden):

    h1T/t1T [121, 512]  = leaky(head_w.T @ stateT + bias) for (b0|b1).
                          Bias (and the ones-row 1.0) enters as a K=1
                          accumulating matmul: bias_row.T x ones_row.
    tUT [121, 2x2560]   : per o, [U(o).T | Wt(o)] @ t1T -> strided-copied to
                          columns b*2560 + (o::10); row 120 = B[y,o]=Wt.t1[y]
    A_T [10, 512]       = WhT_ext.T @ h1T
    out[x, (y,o)]       = h1T.T @ tUT  (+ A via 0/1-indicator matmul as a 2nd
                          PSUM pass) (+ C table via the DVE add that also
                          evacuates PSUM->SBUF)

C[x, y*10+o] = width_table[pos(x,y)] @ Ww.T + cls_b is precomputed on host
(tiny). The output [x, (y,o)] layout makes output DMAs fully contiguous
128-row blocks (row-multiple-of-16 so the qSP HWDGE ring spreads descriptors
across all 16 SDMA engines); inputs ride the qAct ring.
"""

import os
import numpy as np

import concourse.bass as bass
import concourse.bacc as bacc
import concourse.tile as tile
from concourse import mybir
from concourse.bass_utils import run_bass_kernel_spmd

# problem shape (hardcoded per harness contract)
B, S, H = 32, 255, 1024
BS, WD, O = 120, 20, 10
HSZ = 2 * (BS + 1) + WD
SP = 256            # padded S
SP2 = 2 * SP        # paired moving dim
NW = SP * O         # 2560
NREAL = S * O       # 2550
KT = H // 128       # 8
NCORES = 8
BPC = B // NCORES   # 4 batch items per core
NP = BPC // 2       # 2 pairs per core
BSE = BS + 1        # 121

F32 = mybir.dt.float32
F32R = mybir.dt.float32r

_CACHE: dict = {}


def _emit(tc, d):
    """Emit the per-core program. d: dict of DRAM APs."""
    from contextlib import ExitStack

    nc = tc.nc
    AF = mybir.ActivationFunctionType
    ALU = mybir.AluOpType

    with ExitStack() as ctx:
        const = ctx.enter_context(tc.tile_pool(name="const", bufs=1))
        st_pool = ctx.enter_context(tc.tile_pool(name="st", bufs=2))
        ht_pool = ctx.enter_context(tc.tile_pool(name="ht", bufs=2))
        tut_pool = ctx.enter_context(tc.tile_pool(name="tut", bufs=2))
        out_pool = ctx.enter_context(tc.tile_pool(name="outp", bufs=3))
        pp_ht = ctx.enter_context(tc.tile_pool(name="pp_ht", bufs=1, space="PSUM"))
        pp_u = ctx.enter_context(tc.tile_pool(name="pp_u", bufs=2, space="PSUM"))
        pp_s = ctx.enter_context(tc.tile_pool(name="pp_s", bufs=4, space="PSUM"))

        # ---- persistent constants ----
        # head/tail weights carry an extra zero column (-> psum row 120 = 0);
        # biases + the ones-row 1.0 enter via the K=1 bias matmul below.
        # single-partition row first: [1,512] ones | [1,121] hb | [1,121] tb
        sb_row = const.tile([1, SP2 + 2 * BSE], F32R)
        nc.sync.dma_start(sb_row[:], d["row"])
        sb_hw = const.tile([128, KT * BSE], F32R)
        nc.sync.dma_start(sb_hw[:], d["hw"])
        sb_tw = const.tile([128, KT * BSE], F32R)
        nc.sync.dma_start(sb_tw[:], d["tw"])
        # ut: per-o [121, 121] blocks, then 10 cols of WhT_ext.
        sb_ut = const.tile([BSE, O * BSE + 12], F32R)
        nc.sync.dma_start(sb_ut[:], d["ut"])
        sb_ones = sb_row[:, 0:SP2]
        sb_hbr = sb_row[:, SP2:SP2 + BSE]
        sb_tbr = sb_row[:, SP2 + BSE:SP2 + 2 * BSE]
        sb_c0 = const.tile([128, NW], F32)
        sb_c1 = const.tile([128, NW], F32)

        for p in range(NP):
            # ---- load paired stateT (host-packed [128, (kt, b01, y)]) ----
            # two separate half-tiles so kt<4 projections depend only on
            # the first transfer (128-row HWDGE reads on the qAct ring spread
            # across all 16 SDMA engines).
            half = KT * SP2 // 2
            sb_sTa = st_pool.tile([128, half], F32R)
            sb_sTb = st_pool.tile([128, half], F32R)
            nc.scalar.dma_start(sb_sTa[:], d["stateT"][p][:, 0:half])
            nc.scalar.dma_start(sb_sTb[:], d["stateT"][p][:, half:])

            # ---- head/tail projections -> h1T/t1T [121, 512] ----
            ps_h = pp_ht.tile([BSE, SP2], F32)
            ps_t = pp_ht.tile([BSE, SP2], F32)
            for ps, w, br in ((ps_h, sb_hw, sb_hbr), (ps_t, sb_tw, sb_tbr)):
                nc.tensor.matmul(
                    ps[:], lhsT=br, rhs=sb_ones, start=True, stop=False
                )
                for kt in range(KT):
                    st = sb_sTa if kt < 4 else sb_sTb
                    nc.tensor.matmul(
                        ps[:],
                        lhsT=w[:, kt * BSE:(kt + 1) * BSE],
                        rhs=st[:, (kt % 4) * SP2:(kt % 4 + 1) * SP2],
                        start=False,
                        stop=(kt == KT - 1),
                    )
            h1T = ht_pool.tile([BSE, SP2], F32R)
            t1T = ht_pool.tile([BSE, SP2], F32R)
            # u = psum ; leaky = max(u, 0.01u)
            nc.scalar.activation(h1T[:], ps_h[:], AF.Copy)
            nc.vector.scalar_tensor_tensor(
                h1T[:], h1T[:], 0.01, h1T[:], op0=ALU.mult, op1=ALU.max
            )
            nc.scalar.activation(t1T[:], ps_t[:], AF.Copy)
            nc.vector.scalar_tensor_tensor(
                t1T[:], t1T[:], 0.01, t1T[:], op0=ALU.mult, op1=ALU.max
            )
            if p == 0:
                # C loads are first needed by p0's finals; ride the qSP ring,
                # which is otherwise idle until the first output (~+55us).
                nc.sync.dma_start(sb_c0[:], d["cmat"][0:128, :])
                nc.sync.dma_start(sb_c1[:], d["cmat"][128:256, :])

            # ---- tUT [121, 2*2560]: interleaved per-(o,b) copies ----
            tUT = tut_pool.tile([BSE, 2 * NW], F32R)
            for o in range(O):
                ps_u = pp_u.tile([BSE, SP2], F32)
                nc.tensor.matmul(
                    ps_u[:],
                    lhsT=sb_ut[:, o * BSE:(o + 1) * BSE],
                    rhs=t1T[:],
                    start=True,
                    stop=True,
                )
                # strided scatter: tUT[:, bb*NW + (o::10)] <- ps_u[:, bb*SP:+SP]
                for bb in range(2):
                    src = ps_u[:, bb * SP:(bb + 1) * SP]
                    dst = tUT[:, bb * NW + o:bb * NW + NW:O]
                    nc.scalar.activation(dst, src, AF.Copy)

            # ---- finals: out[x, (y,o)] per (b-in-pair, x-tile) ----
            for bb in range(2):
                for xt in range(2):
                    sb_c = sb_c0 if xt == 0 else sb_c1
                    sb_out = out_pool.tile([128, NW], F32)
                    lo = bb * SP + xt * 128
                    for c in range(5):
                        ps_s = pp_s.tile([128, 512], F32)
                        # single pass: the A-term rides inside tUT (folded
                        # into the ut blocks' ones-row on the host).
                        nc.tensor.matmul(
                            ps_s[:],
                            lhsT=h1T[:, lo:lo + 128],
                            rhs=tUT[:, bb * NW + c * 512:bb * NW + (c + 1) * 512],
                            start=True,
                            stop=True,
                        )
                        oc = sb_out[:, c * 512:(c + 1) * 512]
                        cc = sb_c[:, c * 512:(c + 1) * 512]
                        if p == NP - 1 and c >= 3:
                            # drain-phase offload: DVE is the bottleneck at
                            # the tail; route 2/5 chunks via ACT-copy +
                            # GpSimd in-place add (both idle then).
                            nc.scalar.activation(oc, ps_s[:], AF.Copy)
                            nc.gpsimd.tensor_add(oc, oc, cc)
                        else:
                            nc.vector.tensor_add(oc, ps_s[:], cc)
                    # two 64-row (multiple of 16!) contiguous blocks on the
                    # qSP HWDGE ring -> each spreads across the SDMA engines
                    # and the first can start before the last chunk's add.
                    # Row 255 of the padded output absorbs xt=1's garbage.
                    orow = xt * 128
                    nc.sync.dma_start(
                        d["out"][2 * p + bb, orow:orow + 64, :],
                        sb_out[0:64, :],
                    )
                    nc.sync.dma_start(
                        d["out"][2 * p + bb, orow + 64:orow + 128, :],
                        sb_out[64:128, :],
                    )


def build_nc():
    if "nc" in _CACHE:
        return _CACHE["nc"]
    nc = bacc.Bacc(
        "TRN2", target_bir_lowering=False, debug=False, num_devices=NCORES
    )
    d = {}
    d["stateT"] = nc.dram_tensor(
        "stateT", [NP, 128, KT * SP2], F32R, kind="ExternalInput"
    ).ap()
    d["hw"] = nc.dram_tensor("hw", [128, KT * BSE], F32R, kind="ExternalInput").ap()
    d["tw"] = nc.dram_tensor("tw", [128, KT * BSE], F32R, kind="ExternalInput").ap()
    d["ut"] = nc.dram_tensor(
        "ut", [BSE, O * BSE + 12], F32R, kind="ExternalInput"
    ).ap()
    d["row"] = nc.dram_tensor(
        "row", [1, SP2 + 2 * BSE], F32R, kind="ExternalInput"
    ).ap()
    d["cmat"] = nc.dram_tensor("cmat", [SP, NW], F32, kind="ExternalInput").ap()
    d["out"] = nc.dram_tensor("out", [BPC, SP, NW], F32, kind="ExternalOutput").ap()

    with tile.TileContext(nc) as tc:
        _emit(tc, d)
    nc.compile()
    _CACHE["nc"] = nc
    return nc


def prep_inputs(inputs):
    """Host-side constant packing + state transpose. Returns dict of np arrays
    shared across cores (stateT is full-batch; shard before dispatch)."""
    state = np.asarray(inputs["state"], np.float32)
    head_w = np.asarray(inputs["head_w"], np.float32)
    head_b = np.asarray(inputs["head_b"], np.float32)
    tail_w = np.asarray(inputs["tail_w"], np.float32)
    tail_b = np.asarray(inputs["tail_b"], np.float32)
    U = np.asarray(inputs["U"], np.float32)
    width_table = np.asarray(inputs["width_table"], np.float32)
    cls_w = np.asarray(inputs["cls_w"], np.float32)
    cls_b = np.asarray(inputs["cls_b"], np.float32)

    # stateT paired pack: [B/2, 128, (kt, b01, y)], y zero-padded to 256
    stateT = np.zeros((B, H, SP), np.float32)
    stateT[:, :, :S] = state.transpose(0, 2, 1)
    # [B/2, 2, KT, 128, SP] -> [B/2, 128, KT, 2, SP]
    stateT = stateT.reshape(B // 2, 2, KT, 128, SP).transpose(0, 3, 2, 1, 4)
    stateT = np.ascontiguousarray(stateT.reshape(B // 2, 128, KT * SP2))

    hw_sb = np.zeros((128, KT, BSE), np.float32)
    hw_sb[:, :, :BS] = head_w.reshape(KT, 128, BS).transpose(1, 0, 2)
    hw_sb = np.ascontiguousarray(hw_sb.reshape(128, KT * BSE))
    tw_sb = np.zeros((128, KT, BSE), np.float32)
    tw_sb[:, :, :BS] = tail_w.reshape(KT, 128, BS).transpose(1, 0, 2)
    tw_sb = np.ascontiguousarray(tw_sb.reshape(128, KT * BSE))

    # ut blocks + WhT_ext + 2 spare cols
    ut = np.zeros((BSE, O * BSE + 12), np.float32)
    blocks = ut[:, :O * BSE].reshape(BSE, O, BSE)
    blocks[:BS, :, :BS] = U.transpose(2, 0, 1)           # [j, o, i] = U[o,i,j]
    blocks[:, :, BS] = cls_w[:, BS + 1:2 * (BS + 1)].T   # Wt (incl ones coeff)
    # fold the Wh projection (A-term) into the ones-row of each block:
    # t1T row 120 is all-ones, so adding Wh_ext[o, i] here adds A[x, o]
    # (broadcast over y) to the final scores.
    blocks[BS, :, :] += cls_w[:, :BSE]
    ut = np.ascontiguousarray(ut)

    row = np.zeros((1, SP2 + 2 * BSE), np.float32)
    row[0, :S] = 1.0                                     # b0 ones (y=255 -> 0)
    row[0, SP:SP + S] = 1.0                              # b1 ones
    row[0, SP2:SP2 + BS] = head_b
    row[0, SP2 + BS] = 1.0                               # ones-row constant
    row[0, SP2 + BSE:SP2 + BSE + BS] = tail_b
    row[0, SP2 + BSE + BS] = 1.0

    pos = np.arange(S)[None, :] - np.arange(S)[:, None] + 1
    pos = pos * (pos > 0)
    wproj = width_table @ cls_w[:, 2 * (BS + 1):].T + cls_b   # [256, 10]
    cmat = np.zeros((SP, NW), np.float32)
    cmat[:S, :NREAL] = wproj[pos].reshape(S, NREAL)

    return {
        "stateT": stateT,
        "hw": hw_sb,
        "tw": tw_sb,
        "ut": ut,
        "row": row,
        "cmat": cmat,
    }


def run(inputs, trace=False, trace_kwargs=None):
    nc = build_nc()
    full = prep_inputs(inputs)
    shared = {k: v for k, v in full.items() if k != "stateT"}
    in_maps = []
    for c in range(NCORES):
        m = dict(shared)
        m["stateT"] = np.ascontiguousarray(full["stateT"][c * NP:(c + 1) * NP])
        in_maps.append(m)
    res = run_bass_kernel_spmd(
        nc,
        in_maps,
        core_ids=list(range(NCORES)),
        trace=trace,
        **(trace_kwargs or {}),
    )
    out = np.concatenate([r["out"] for r in res.results], axis=0)
    out = out[:, :S, :NREAL].reshape(B, S, S, O)
    return out, res


def kernel(**inputs):
    out, _ = run(inputs, trace=False)
    return out


if __name__ == "__main__":
    build_nc()
    print("build ok")
